# revision 10
# baseline (speedup 1.0000x reference)
"""Trainium2 Bass kernel for nn_MultiHeadedLinrec (linear attention).

Math (per batch element, reference semantics):
    q = elu(x_q @ Wq.T + bq)    [S, E] viewed as [S, H, d]
    k = elu(x_k @ Wk.T + bk)
    v = x_v @ Wv.T + bv
    k <- k / (||k||_seq * sqrt(S))     (per (h, d) column norm over S)
    q <- q / (||q||_d   * sqrt(d))     (per (s, h) row norm over d)
    scores_h = k_h^T @ v_h             [d, d]
    out = concat_h(q_h @ scores_h) @ Wo.T + bo

Kernel strategy (one NeuronCore per batch element, 8 cores data-parallel):
  Phase A (stream S in 128-row tiles): transpose x_k/x_v on PE, project to
    natural layout [s, e], ELU(k), accumulate per-head scoresT = v_h^T k_h
    and column sums of k^2 on the PE.
  Phase B: fold k-norm + scores + Wo into one fused weight
    W2[i, o] = (scores @ Wo.T)[i, o] / (knorm[i] * sqrt(S)),
    built as 8 block-diagonal 128x128 matmuls against WoT tiles.
  Phase C (stream S in 512-col blocks): transposed q projection (qT layout),
    ELU with per-partition bias, row-norms via block-ones matmul + PE
    broadcast, scale, then out = qnT.T @ W2 + bo in natural layout.

All large matmuls run in float32r (TF32-like, ~1.2e-4 rel rounding, full
bf16-rate on the PE for moving dim >= 256).  scoresT accumulation runs in
exact fp32 (N=64 is 4 cyc/row for either dtype).

This walrus build only supports ONE sync wait per instruction; Tile emits
multi-wait instructions, so we legalize the BIR JSON by hoisting extra waits
onto inserted NoOps (see _legalize_sync_json).
"""

import json

import numpy as np

import concourse.bass as bass
import concourse.mybir as mybir
import concourse.tile as tile
from concourse import masks
from concourse.bass_utils import run_bass_kernel_spmd

dt = mybir.dt
AF = mybir.ActivationFunctionType
ALU = mybir.AluOpType

P = 128
E = 1024
H = 16
D = 64
N_CORES = 8
EC = E // P  # 8 chunks of 128 along the embedding dim
SBLK = 512  # phase-C s-block


# --------------------------------------------------------------------------
# BIR sync legalization: max one wait / one update per instruction.
# --------------------------------------------------------------------------
def _legalize_sync_json(bir_json: bytes) -> bytes:
    m = json.loads(bir_json)
    counter = [0]

    def fresh():
        counter[0] += 1
        return f"I-synclift-{counter[0]}"

    for f in m["functions"]:
        for blk in f["blocks"]:
            out = []
            for ins in blk["instructions"]:
                si = ins.get("sync_info")
                if not si:
                    out.append(ins)
                    continue
                waits = si.get("on_wait") or []
                updates = si.get("on_update") or []
                if len(waits) <= 1 and len(updates) <= 1:
                    out.append(ins)
                    continue
                eng = ins.get("engine")
                dbg = ins.get("debug")
                for w in waits[:-1]:
                    out.append(
                        {
                            "debug": dbg,
                            "engine": eng,
                            "ins": [],
                            "name": fresh(),
                            "opcode": "NoOp",
                            "outs": [],
                            "sync_info": {"on_update": [], "on_wait": [w]},
                        }
                    )
                si["on_wait"] = waits[-1:]
                post = [
                    {
                        "debug": dbg,
                        "engine": eng,
                        "ins": [],
                        "name": fresh(),
                        "opcode": "NoOp",
                        "outs": [],
                        "sync_info": {"on_update": [u], "on_wait": []},
                    }
                    for u in updates[1:]
                ]
                si["on_update"] = updates[:1]
                out.append(ins)
                out.extend(post)
            blk["instructions"] = out
    return json.dumps(m).encode()


def _patch_bass(nc):
    orig = nc.to_json_bytes

    def patched():
        return _legalize_sync_json(orig())

    nc.to_json_bytes = patched
    return nc


# --------------------------------------------------------------------------
# Kernel builder
# --------------------------------------------------------------------------
def build(S: int = 4096):
    ST = S // P  # number of 128-row s-tiles
    NBLK = S // SBLK  # number of phase-C blocks
    JB = SBLK // P  # s-tiles per block (4)

    nc = bass.Bass(trn_type="TRN2", target_bir_lowering=False, debug=False)

    xq = nc.dram_tensor("xq", [S, E], dt.float32, kind="ExternalInput").ap()
    xk = nc.dram_tensor("xk", [S, E], dt.float32, kind="ExternalInput").ap()
    xv = nc.dram_tensor("xv", [S, E], dt.float32, kind="ExternalInput").ap()
    Wq = nc.dram_tensor("Wq", [E, E], dt.float32, kind="ExternalInput").ap()
    Wk = nc.dram_tensor("Wk", [E, E], dt.float32, kind="ExternalInput").ap()
    Wv = nc.dram_tensor("Wv", [E, E], dt.float32, kind="ExternalInput").ap()
    Wo = nc.dram_tensor("Wo", [E, E], dt.float32, kind="ExternalInput").ap()
    bq = nc.dram_tensor("bq", [1, E], dt.float32, kind="ExternalInput").ap()
    bk = nc.dram_tensor("bk", [1, E], dt.float32, kind="ExternalInput").ap()
    bv = nc.dram_tensor("bv", [1, E], dt.float32, kind="ExternalInput").ap()
    bo = nc.dram_tensor("bo", [1, E], dt.float32, kind="ExternalInput").ap()
    out = nc.dram_tensor("out", [S, E], dt.float32, kind="ExternalOutput").ap()

    f32 = dt.float32
    f32r = dt.float32r

    with tile.TileContext(nc) as tc:
        with (
            tc.tile_pool(name="consts", bufs=1) as consts,
            tc.tile_pool(name="small", bufs=1) as small,
            tc.tile_pool(name="drpool", bufs=1, space="DRAM") as drpool,
        ):
            # ---------------- constants ----------------
            ident = consts.tile([P, P], f32, name="ident")
            masks.make_identity(nc, ident[:])

            zero128 = consts.tile([P, P], f32, name="zero128")
            nc.vector.memset(zero128[:], 0.0)

            # f32 staging for all f32r constants (the verifier requires
            # fp32r-matmul operands to come from a rounding op: tensor_copy)
            ones_st = consts.tile([1, P], f32, name="ones_st")
            nc.vector.memset(ones_st[:], 1.0)
            ones_1x128 = consts.tile([1, P], f32r, name="ones_1x128")
            nc.vector.tensor_copy(ones_1x128[:], ones_st[:])

            ones2 = []
            for h in range(2):
                st = consts.tile([P, 2], f32, name=f"ones2_st_{h}")
                nc.vector.memset(st[:], 0.0)
                nc.vector.memset(st[:, h : h + 1], 1.0)
                t = consts.tile([P, 2], f32r, name=f"ones2_{h}")
                nc.vector.tensor_copy(t[:], st[:])
                ones2.append(t)

            blockones = []
            blockones_st = []
            for c in range(EC):
                st = consts.tile([P, H], f32, name=f"blockones_st_{c}")
                nc.vector.memset(st[:], 0.0)
                nc.vector.memset(st[0:D, 2 * c : 2 * c + 1], 1.0)
                nc.vector.memset(st[D:P, 2 * c + 1 : 2 * c + 2], 1.0)
                blockones_st.append(st)
                tt = consts.tile([P, H], f32r, name=f"blockones_{c}")
                nc.vector.tensor_copy(tt[:], st[:])
                blockones.append(tt)

            # blockpick_c = blockones_c^T via PE transpose (f32), then round
            blockpick = []
            with tc.tile_pool(name="bp_ps", bufs=2, space="PSUM") as bp_ps:
                for c in range(EC):
                    pt = bp_ps.tile([H, P], f32, name="bp_ps")
                    nc.tensor.transpose(pt[:], blockones_st[c][:], ident[:])
                    tt = consts.tile([H, P], f32r, name=f"blockpick_{c}")
                    nc.vector.tensor_copy(tt[:], pt[:])
                    blockpick.append(tt)

            # ---------------- biases ----------------
            with tc.tile_pool(name="brow_stage", bufs=2) as stage_pool:
                def load_row_r(name, src):
                    stage = stage_pool.tile([1, E], f32, name="brow_stage")
                    nc.sync.dma_start(stage[:], src)
                    row = small.tile([1, E], f32r, name=f"{name}_r")
                    nc.vector.tensor_copy(row[:], stage[:])
                    return row

                bk_row = load_row_r("bk", bk)
                bv_row = load_row_r("bv", bv)
                bo_row = load_row_r("bo", bo)

            bq_col = small.tile([P, EC], f32, name="bq_col")
            nc.sync.dma_start(bq_col[:], bq.rearrange("1 (t p) -> p t", p=P))

            with tc.tile_pool(name="bias_ps", bufs=2, space="PSUM") as bias_ps:
                def bcast_row(row_r, name):
                    full = small.tile([P, E], f32, name=f"{name}_bcast")
                    for h in range(2):
                        pt = bias_ps.tile([P, 512], f32, name="bias_ps")
                        nc.tensor.matmul(
                            pt[:],
                            ones_1x128[:],
                            row_r[:, h * 512 : (h + 1) * 512],
                            start=True,
                            stop=True,
                        )
                        nc.vector.tensor_copy(full[:, h * 512 : (h + 1) * 512], pt[:])
                    return full

                bo_bcast = bcast_row(bo_row, "bo")

            # WT[i, o] = W[o, i]; stored as EC tiles [128(i), E(o)] in f32r
            def load_wt(W, name, dst_pool, wstage_pool, tr_ps, tr_tag):
                tiles = [
                    dst_pool.tile([P, E], f32r, name=f"{name}T_{c}")
                    for c in range(EC)
                ]
                for r in range(EC):
                    wnat = wstage_pool.tile([P, E], f32, name="wstage")
                    nc.sync.dma_start(wnat[:], W[r * P : (r + 1) * P, :])
                    for h in range(2):
                        pt = tr_ps.tile([P, 512], f32, name=tr_tag)
                        for c4 in range(4):
                            c = h * 4 + c4
                            nc.tensor.transpose(
                                pt[:, c4 * P : (c4 + 1) * P],
                                wnat[:, c * P : (c + 1) * P],
                                ident[:],
                            )
                        for c4 in range(4):
                            c = h * 4 + c4
                            nc.vector.tensor_copy(
                                tiles[c][:, r * P : (r + 1) * P],
                                pt[:, c4 * P : (c4 + 1) * P],
                            )
                return tiles

            bd_st = []
            for pr in range(8):
                s_t = small.tile([P, P], f32, name=f"bd_st_{pr}")
                nc.vector.memset(s_t[:], 0.0)
                bd_st.append(s_t)

            # ================= PHASE A ====================================
            with (
                tc.tile_pool(name="wts_kv", bufs=1) as wts_kv,
                tc.tile_pool(name="a_in", bufs=2) as a_in,
                tc.tile_pool(name="a_xt", bufs=2) as a_xt,
                tc.tile_pool(name="a_act", bufs=2) as a_act,
                tc.tile_pool(name="a_tr_ps", bufs=2, space="PSUM") as a_tr_ps,
                tc.tile_pool(name="a_pj_ps", bufs=4, space="PSUM") as a_pj_ps,
                tc.tile_pool(name="a_sc_ps", bufs=1, space="PSUM") as a_sc_ps,
                tc.tile_pool(name="a_ksq_ps", bufs=1, space="PSUM") as a_ksq_ps,
            ):
                WkT = load_wt(Wk, "Wk", wts_kv, a_in, a_tr_ps, "a_tr")
                WvT = load_wt(Wv, "Wv", wts_kv, a_in, a_tr_ps, "a_tr")

                scores_ps = a_sc_ps.tile([P, 8 * D], f32, name="scores_ps")
                for qtr in range(4):
                    nc.tensor.matmul(
                        scores_ps[:, qtr * P : (qtr + 1) * P],
                        zero128[:],
                        zero128[:],
                        start=True,
                        stop=True,
                        skip_group_check=True,
                    )
                ksq_ps = a_ksq_ps.tile([2, 512], f32, name="ksq_ps")

                def transpose_in(x_src, it, name):
                    xnat = a_in.tile([P, E], f32, name=f"{name}_nat")
                    nc.sync.dma_start(xnat[:], x_src[it * P : (it + 1) * P, :])
                    xt = a_xt.tile([P, E], f32r, name=f"{name}_T")
                    for h in range(2):
                        pt = a_tr_ps.tile([P, 512], f32, name="a_tr")
                        for c4 in range(4):
                            c = h * 4 + c4
                            nc.tensor.transpose(
                                pt[:, c4 * P : (c4 + 1) * P],
                                xnat[:, c * P : (c + 1) * P],
                                ident[:],
                            )
                        nc.vector.tensor_copy(xt[:, h * 512 : (h + 1) * 512], pt[:])
                    return xt

                def project_nat(xt, WT, brow, name):
                    halves = []
                    for h in range(2):
                        pj = a_pj_ps.tile([P, 512], f32, name="pj")
                        for c in range(EC):
                            nc.tensor.matmul(
                                pj[:],
                                xt[:, c * P : (c + 1) * P],
                                WT[c][:, h * 512 : (h + 1) * 512],
                                start=(c == 0),
                                stop=False,
                            )
                        nc.tensor.matmul(
                            pj[:],
                            ones_1x128[:],
                            brow[:, h * 512 : (h + 1) * 512],
                            start=False,
                            stop=True,
                        )
                        halves.append(pj)
                    return halves

                for it in range(ST):
                    xkT = transpose_in(xk, it, "xk")
                    kp = project_nat(xkT, WkT, bk_row, "k")
                    k_sb = a_act.tile([P, E], f32, name="k_sb")
                    for h in range(2):
                        sl = slice(h * 512, (h + 1) * 512)
                        r_sb = a_act.tile([P, 512], f32, name="kr_sb")
                        t_sb = a_act.tile([P, 512], f32, name="kt_sb")
                        e_sb = a_act.tile([P, 512], f32, name="ke_sb")
                        nc.scalar.activation(r_sb[:], kp[h][:], AF.Relu)
                        nc.vector.tensor_scalar(
                            t_sb[:], kp[h][:], 0.0, None, ALU.min
                        )
                        nc.scalar.activation(e_sb[:], t_sb[:], AF.Exp)
                        nc.vector.scalar_tensor_tensor(
                            k_sb[:, sl], e_sb[:], -1.0, r_sb[:], ALU.add, ALU.add
                        )
                        k2_sb = a_act.tile([P, 512], f32r, name="k2_sb")
                        nc.scalar.activation(k2_sb[:], k_sb[:, sl], AF.Square)
                        nc.tensor.matmul(
                            ksq_ps[:],
                            ones2[h][:],
                            k2_sb[:],
                            start=(it == 0 and h == 0),
                            stop=(it == ST - 1 and h == 1),
                        )

                    xvT = transpose_in(xv, it, "xv")
                    vp = project_nat(xvT, WvT, bv_row, "v")
                    v_sb = a_act.tile([P, E], f32, name="v_sb")
                    for h in range(2):
                        sl = slice(h * 512, (h + 1) * 512)
                        nc.vector.tensor_copy(v_sb[:, sl], vp[h][:])

                    for pr in range(8):
                        for sub in range(2):
                            hh = 2 * pr + sub
                            nc.tensor.matmul(
                                scores_ps[sub * D : (sub + 1) * D,
                                          pr * D : (pr + 1) * D],
                                v_sb[:, hh * D : (hh + 1) * D],
                                k_sb[:, hh * D : (hh + 1) * D],
                                start=False,
                                stop=(it == ST - 1 and pr == 7 and sub == 1),
                                skip_group_check=True,
                            )

                # -- extract scoresT + ksumsq while phase-A psum still alive
                ksq_sb = small.tile([2, 512], f32, name="ksq_sb")
                nc.vector.tensor_copy(ksq_sb[:], ksq_ps[:])
                ksq_dram = drpool.tile([1, E], f32, name="ksq_dram")
                nc.sync.dma_start(
                    ksq_dram[:].rearrange("1 (h n) -> h n", h=2), ksq_sb[:]
                )
                kcol = small.tile([P, EC], f32, name="kcol")
                nc.sync.dma_start(
                    kcol[:], ksq_dram[:].rearrange("1 (t p) -> p t", p=P)
                )
                knorm = small.tile([P, EC], f32, name="knorm")
                nc.scalar.activation(knorm[:], kcol[:], AF.Sqrt, scale=float(S))
                invk = small.tile([P, EC], f32, name="invk")
                nc.vector.reciprocal(invk[:], knorm[:])

                bd = []
                for pr in range(8):
                    nc.vector.tensor_copy(
                        bd_st[pr][0:D, 0:D], scores_ps[0:D, pr * D : (pr + 1) * D]
                    )
                    nc.vector.tensor_copy(
                        bd_st[pr][D:P, D:P], scores_ps[D:P, pr * D : (pr + 1) * D]
                    )
                    bd_t = small.tile([P, P], f32r, name=f"bd_{pr}")
                    nc.vector.tensor_copy(bd_t[:], bd_st[pr][:])
                    bd.append(bd_t)

            # ================= PHASE B: W2 ================================
            w2scope = tc.tile_pool(name="w2pool", bufs=1)
            w2pool = w2scope.__enter__()
            W2 = [w2pool.tile([P, E], f32r, name=f"W2_{c}") for c in range(EC)]
            with (
                tc.tile_pool(name="wts_o", bufs=1) as wts_o,
                tc.tile_pool(name="b_stage", bufs=2) as b_stage,
                tc.tile_pool(name="b_tr_ps", bufs=2, space="PSUM") as b_tr_ps,
                tc.tile_pool(name="b_ps", bufs=4, space="PSUM") as b_ps,
            ):
                WoT = load_wt(Wo, "Wo", wts_o, b_stage, b_tr_ps, "b_tr")
                for c in range(EC):
                    for h in range(2):
                        w2p = b_ps.tile([P, 512], f32, name="w2_ps")
                        nc.tensor.matmul(
                            w2p[:],
                            bd[c][:],
                            WoT[c][:, h * 512 : (h + 1) * 512],
                            start=True,
                            stop=True,
                        )
                        nc.vector.tensor_scalar(
                            W2[c][:, h * 512 : (h + 1) * 512],
                            w2p[:],
                            invk[:, c : c + 1],
                            None,
                            ALU.mult,
                        )

            # ================= PHASE C: q pass ============================
            with (
                tc.tile_pool(name="wts_q", bufs=1) as wts_q,
                tc.tile_pool(name="c_in", bufs=2) as c_in,
                tc.tile_pool(name="c_xt", bufs=1) as c_xt,
                tc.tile_pool(name="c_qt", bufs=1) as c_qt,
                tc.tile_pool(name="c_tmp", bufs=2) as c_tmp,
                tc.tile_pool(name="c_out", bufs=2) as c_out,
                tc.tile_pool(name="c_tr_ps", bufs=2, space="PSUM") as c_tr_ps,
                tc.tile_pool(name="c_pj_ps", bufs=2, space="PSUM") as c_pj_ps,
                tc.tile_pool(name="c_ss_ps", bufs=1, space="PSUM") as c_ss_ps,
                tc.tile_pool(name="c_qb_ps", bufs=1, space="PSUM") as c_qb_ps,
                tc.tile_pool(name="c_fin_ps", bufs=2, space="PSUM") as c_fin_ps,
            ):
                WqT = load_wt(Wq, "Wq", wts_q, c_in, c_tr_ps, "c_tr")

                for blk_i in range(NBLK):
                    s0 = blk_i * SBLK
                    xqT = [
                        c_xt.tile([P, SBLK], f32r, name=f"xqT_{c}")
                        for c in range(EC)
                    ]
                    for j in range(JB):
                        xnat = c_in.tile([P, E], f32, name="xq_nat")
                        nc.sync.dma_start(
                            xnat[:], xq[s0 + j * P : s0 + (j + 1) * P, :]
                        )
                        for h in range(2):
                            pt = c_tr_ps.tile([P, 512], f32, name="c_tr")
                            for c4 in range(4):
                                c = h * 4 + c4
                                nc.tensor.transpose(
                                    pt[:, c4 * P : (c4 + 1) * P],
                                    xnat[:, c * P : (c + 1) * P],
                                    ident[:],
                                )
                            for c4 in range(4):
                                c = h * 4 + c4
                                nc.vector.tensor_copy(
                                    xqT[c][:, j * P : (j + 1) * P],
                                    pt[:, c4 * P : (c4 + 1) * P],
                                )

                    qss_ps = c_ss_ps.tile([H, SBLK], f32, name="qss_ps")
                    qt_tiles = []
                    for ot in range(EC):
                        pj = c_pj_ps.tile([P, SBLK], f32, name="q_pj")
                        for c in range(EC):
                            nc.tensor.matmul(
                                pj[:],
                                WqT[c][:, ot * P : (ot + 1) * P],
                                xqT[c][:],
                                start=(c == 0),
                                stop=(c == EC - 1),
                            )
                        r_sb = c_tmp.tile([P, SBLK], f32, name="qr_sb")
                        t_sb = c_tmp.tile([P, SBLK], f32, name="qt_sb")
                        e_sb = c_tmp.tile([P, SBLK], f32, name="qe_sb")
                        qt_ = c_qt.tile([P, SBLK], f32, name=f"qt_{ot}")
                        nc.scalar.activation(
                            r_sb[:], pj[:], AF.Relu, bias=bq_col[:, ot : ot + 1]
                        )
                        nc.vector.tensor_scalar(
                            t_sb[:], pj[:], bq_col[:, ot : ot + 1], 0.0,
                            ALU.add, ALU.min,
                        )
                        nc.scalar.activation(e_sb[:], t_sb[:], AF.Exp)
                        nc.vector.scalar_tensor_tensor(
                            qt_[:], e_sb[:], -1.0, r_sb[:], ALU.add, ALU.add
                        )
                        qt_tiles.append(qt_)
                        q2 = c_tmp.tile([P, SBLK], f32r, name="q2_sb")
                        nc.scalar.activation(q2[:], qt_[:], AF.Square)
                        nc.tensor.matmul(
                            qss_ps[:],
                            blockones[ot][:],
                            q2[:],
                            start=(ot == 0),
                            stop=(ot == EC - 1),
                        )

                    qss_sb = c_tmp.tile([H, SBLK], f32, name="qss_sb")
                    nc.scalar.activation(qss_sb[:], qss_ps[:], AF.Sqrt,
                                         scale=float(D))
                    invq = c_tmp.tile([H, SBLK], f32, name="invq")
                    nc.vector.reciprocal(invq[:], qss_sb[:])
                    invq_r = c_tmp.tile([H, SBLK], f32r, name="invq_r")
                    nc.vector.tensor_copy(invq_r[:], invq[:])

                    # broadcast + in-place scale (qt tile becomes f32r qn)
                    qn_tiles = []
                    for ot in range(EC):
                        qb = c_qb_ps.tile([P, SBLK], f32, name="qb_ps")
                        nc.tensor.matmul(
                            qb[:], blockpick[ot][:], invq_r[:],
                            start=True, stop=True,
                        )
                        qn = c_qt.tile([P, SBLK], f32r, name=f"qn_{ot}")
                        nc.vector.tensor_tensor(
                            qn[:], qt_tiles[ot][:], qb[:], ALU.mult
                        )
                        qn_tiles.append(qn)

                    for j in range(JB):
                        o_sb = c_out.tile([P, E], f32, name="o_sb")
                        for h in range(2):
                            fin = c_fin_ps.tile([P, 512], f32, name="fin_ps")
                            for c in range(EC):
                                nc.tensor.matmul(
                                    fin[:],
                                    qn_tiles[c][:, j * P : (j + 1) * P],
                                    W2[c][:, h * 512 : (h + 1) * 512],
                                    start=(c == 0),
                                    stop=(c == EC - 1),
                                )
                            sl = slice(h * 512, (h + 1) * 512)
                            nc.vector.scalar_tensor_tensor(
                                o_sb[:, sl], fin[:], 0.0, bo_bcast[:, sl],
                                ALU.add, ALU.add,
                            )
                        nc.sync.dma_start(
                            out[s0 + j * P : s0 + (j + 1) * P, :], o_sb[:]
                        )
            w2scope.__exit__(None, None, None)

    _patch_bass(nc)
    return nc


# --------------------------------------------------------------------------
# Host wrapper
# --------------------------------------------------------------------------
_NC_CACHE = {}


def _get_nc(S):
    if S not in _NC_CACHE:
        _NC_CACHE[S] = build(S)
    return _NC_CACHE[S]


def make_in_maps(query, key, value, Wq, bq, Wk, bk, Wv, bv, Wo, bo):
    query = np.asarray(query, np.float32)
    key = np.asarray(key, np.float32)
    value = np.asarray(value, np.float32)
    B = query.shape[0]
    shared = {
        "Wq": np.ascontiguousarray(np.asarray(Wq, np.float32)),
        "Wk": np.ascontiguousarray(np.asarray(Wk, np.float32)),
        "Wv": np.ascontiguousarray(np.asarray(Wv, np.float32)),
        "Wo": np.ascontiguousarray(np.asarray(Wo, np.float32)),
        "bq": np.ascontiguousarray(np.asarray(bq, np.float32).reshape(1, E)),
        "bk": np.ascontiguousarray(np.asarray(bk, np.float32).reshape(1, E)),
        "bv": np.ascontiguousarray(np.asarray(bv, np.float32).reshape(1, E)),
        "bo": np.ascontiguousarray(np.asarray(bo, np.float32).reshape(1, E)),
    }
    return [
        {
            "xq": np.ascontiguousarray(query[c]),
            "xk": np.ascontiguousarray(key[c]),
            "xv": np.ascontiguousarray(value[c]),
            **shared,
        }
        for c in range(B)
    ]


def kernel(query, key, value, Wq, bq, Wk, bk, Wv, bv, Wo, bo):
    query = np.asarray(query, np.float32)
    B, S, E_ = query.shape
    assert E_ == E and B == N_CORES
    in_maps = make_in_maps(query, key, value, Wq, bq, Wk, bk, Wv, bv, Wo, bo)
    nc = _get_nc(S)
    res = run_bass_kernel_spmd(nc, in_maps, core_ids=list(range(N_CORES)))
    return np.stack([res.results[c]["out"] for c in range(B)])


# revision 22
# speedup vs baseline: 94996.8398x; 94996.8398x over previous
"""Trainium2 Bass kernel for nn_MultiHeadedLinrec (linear attention).

Math (per batch element, reference semantics):
    q = elu(x_q @ Wq.T + bq)    [S, E] viewed as [S, H, d]
    k = elu(x_k @ Wk.T + bk)
    v = x_v @ Wv.T + bv
    k <- k / (||k||_seq * sqrt(S))     (per (h, d) column norm over S)
    q <- q / (||q||_d   * sqrt(d))     (per (s, h) row norm over d)
    scores_h = k_h^T @ v_h             [d, d]
    out = concat_h(q_h @ scores_h) @ Wo.T + bo

Kernel strategy (one NeuronCore per batch element, 8 cores data-parallel):
  Phase A (stream S in 128-row tiles): transpose x_k/x_v on PE, project to
    natural layout [s, e], ELU(k), accumulate per-head scoresT = v_h^T k_h
    and column sums of k^2 on the PE.
  Phase B: fold k-norm + scores + Wo into one fused weight
    W2[i, o] = (scores @ Wo.T)[i, o] / (knorm[i] * sqrt(S)),
    built as 8 block-diagonal 128x128 matmuls against WoT tiles.
  Phase C (stream S in 512-col blocks): transposed q projection (qT layout),
    ELU with per-partition bias, row-norms via block-ones matmul + PE
    broadcast, scale, then out = qnT.T @ W2 + bo in natural layout.

All large matmuls run in float32r (TF32-like, ~1.2e-4 rel rounding, full
bf16-rate on the PE for moving dim >= 256).  scoresT accumulation runs in
exact fp32 (N=64 is 4 cyc/row for either dtype).

This walrus build only supports ONE sync wait per instruction; Tile emits
multi-wait instructions, so we legalize the BIR JSON by hoisting extra waits
onto inserted NoOps (see _legalize_sync_json).
"""

import json

import numpy as np

import concourse.bass as bass
import concourse.mybir as mybir
import concourse.tile as tile
from concourse import masks
from concourse.bass_utils import run_bass_kernel_spmd

dt = mybir.dt
AF = mybir.ActivationFunctionType
ALU = mybir.AluOpType

P = 128
E = 1024
H = 16
D = 64
N_CORES = 8
EC = E // P  # 8 chunks of 128 along the embedding dim
SBLK = 512  # phase-C s-block


# --------------------------------------------------------------------------
# BIR sync legalization: max one wait / one update per instruction.
# --------------------------------------------------------------------------
def _legalize_sync_json(bir_json: bytes) -> bytes:
    m = json.loads(bir_json)
    counter = [0]

    def fresh():
        counter[0] += 1
        return f"I-synclift-{counter[0]}"

    for f in m["functions"]:
        for blk in f["blocks"]:
            out = []
            for ins in blk["instructions"]:
                si = ins.get("sync_info")
                if not si:
                    out.append(ins)
                    continue
                waits = si.get("on_wait") or []
                updates = si.get("on_update") or []
                if len(waits) <= 1 and len(updates) <= 1:
                    out.append(ins)
                    continue
                eng = ins.get("engine")
                dbg = ins.get("debug")
                for w in waits[:-1]:
                    out.append(
                        {
                            "debug": dbg,
                            "engine": eng,
                            "ins": [],
                            "name": fresh(),
                            "opcode": "NoOp",
                            "outs": [],
                            "sync_info": {"on_update": [], "on_wait": [w]},
                        }
                    )
                si["on_wait"] = waits[-1:]
                post = [
                    {
                        "debug": dbg,
                        "engine": eng,
                        "ins": [],
                        "name": fresh(),
                        "opcode": "NoOp",
                        "outs": [],
                        "sync_info": {"on_update": [u], "on_wait": []},
                    }
                    for u in updates[1:]
                ]
                si["on_update"] = updates[:1]
                out.append(ins)
                out.extend(post)
            blk["instructions"] = out
    return json.dumps(m).encode()


def _patch_bass(nc):
    orig = nc.to_json_bytes

    def patched():
        return _legalize_sync_json(orig())

    nc.to_json_bytes = patched
    return nc


# --------------------------------------------------------------------------
# Kernel builder
# --------------------------------------------------------------------------
def build(S: int = 4096, with_bias: bool = True, cfg: dict | None = None):
    cfg = cfg or {}
    ST = S // P  # number of 128-row s-tiles
    NBLK = S // SBLK  # number of phase-C blocks
    JB = SBLK // P  # s-tiles per block (4)

    nc = bass.Bass(trn_type="TRN2", target_bir_lowering=False, debug=False)

    xq = nc.dram_tensor("xq", [S, E], dt.float32, kind="ExternalInput").ap()
    xk = nc.dram_tensor("xk", [S, E], dt.float32, kind="ExternalInput").ap()
    xv = nc.dram_tensor("xv", [S, E], dt.float32, kind="ExternalInput").ap()
    Wq = nc.dram_tensor("Wq", [E, E], dt.float32, kind="ExternalInput").ap()
    Wk = nc.dram_tensor("Wk", [E, E], dt.float32, kind="ExternalInput").ap()
    Wv = nc.dram_tensor("Wv", [E, E], dt.float32, kind="ExternalInput").ap()
    Wo = nc.dram_tensor("Wo", [E, E], dt.float32, kind="ExternalInput").ap()
    bq = nc.dram_tensor("bq", [1, E], dt.float32, kind="ExternalInput").ap()
    bk = nc.dram_tensor("bk", [1, E], dt.float32, kind="ExternalInput").ap()
    bv = nc.dram_tensor("bv", [1, E], dt.float32, kind="ExternalInput").ap()
    bo = nc.dram_tensor("bo", [1, E], dt.float32, kind="ExternalInput").ap()
    out = nc.dram_tensor("out", [S, E], dt.float32, kind="ExternalOutput").ap()

    f32 = dt.float32
    f32r = dt.float32r

    with tile.TileContext(nc) as tc:
        with (
            tc.tile_pool(name="consts", bufs=1) as consts,
            tc.tile_pool(name="small", bufs=1) as small,
            tc.tile_pool(name="drpool", bufs=1, space="DRAM") as drpool,
        ):
            # ---------------- constants ----------------
            ident = consts.tile([P, P], f32, name="ident")
            masks.make_identity(nc, ident[:])

            zero128 = consts.tile([P, P], f32, name="zero128")
            nc.vector.memset(zero128[:], 0.0)

            # f32 staging for all f32r constants (the verifier requires
            # fp32r-matmul operands to come from a rounding op: tensor_copy)
            ones_st = consts.tile([1, P], f32, name="ones_st")
            nc.vector.memset(ones_st[:], 1.0)
            ones_1x128 = consts.tile([1, P], f32r, name="ones_1x128")
            nc.vector.tensor_copy(ones_1x128[:], ones_st[:])

            blockones = []
            blockones_st = []
            for c in range(EC):
                st = consts.tile([P, H], f32, name=f"blockones_st_{c}")
                nc.vector.memset(st[:], 0.0)
                nc.vector.memset(st[0:D, 2 * c : 2 * c + 1], 1.0)
                nc.vector.memset(st[D:P, 2 * c + 1 : 2 * c + 2], 1.0)
                blockones_st.append(st)
                tt = consts.tile([P, H], f32r, name=f"blockones_{c}")
                nc.vector.tensor_copy(tt[:], st[:])
                blockones.append(tt)

            # blockpick_c = blockones_c^T via PE transpose (f32), then round
            blockpick = []
            with tc.tile_pool(name="bp_ps", bufs=2, space="PSUM") as bp_ps:
                for c in range(EC):
                    pt = bp_ps.tile([H, P], f32, name="bp_ps")
                    nc.tensor.transpose(pt[:], blockones_st[c][:], ident[:])
                    tt = consts.tile([H, P], f32r, name=f"blockpick_{c}")
                    nc.vector.tensor_copy(tt[:], pt[:])
                    blockpick.append(tt)

            # ---------------- biases ----------------
            rows_scope = tc.tile_pool(name="rows", bufs=1)
            rows_pool = rows_scope.__enter__()
            bk_row = bv_row = bo_bcast = bq_col = None
            if with_bias:
                with tc.tile_pool(name="brow_stage", bufs=2) as stage_pool:
                    def load_row_r(name, src):
                        stage = stage_pool.tile([1, E], f32, name="brow_stage")
                        nc.sync.dma_start(stage[:], src)
                        row = rows_pool.tile([1, E], f32r, name=f"{name}_r")
                        nc.vector.tensor_copy(row[:], stage[:])
                        return row

                    bk_row = load_row_r("bk", bk)
                    bv_row = load_row_r("bv", bv)
                    bo_row = load_row_r("bo", bo)

                bq_col = small.tile([P, EC], f32, name="bq_col")
                nc.sync.dma_start(bq_col[:], bq.rearrange("1 (t p) -> p t", p=P))

                with tc.tile_pool(name="bias_ps", bufs=2, space="PSUM") as bias_ps:
                    def bcast_row(row_r, name):
                        full = small.tile([P, E], f32, name=f"{name}_bcast")
                        for h in range(2):
                            pt = bias_ps.tile([P, 512], f32, name="bias_ps")
                            nc.tensor.matmul(
                                pt[:],
                                ones_1x128[:],
                                row_r[:, h * 512 : (h + 1) * 512],
                                start=True,
                                stop=True,
                            )
                            nc.vector.tensor_copy(
                                full[:, h * 512 : (h + 1) * 512], pt[:]
                            )
                        return full

                    bo_bcast = bcast_row(bo_row, "bo")

            # WT[i, o] = W[o, i]; stored as EC tiles [128(i), E(o)] in f32r
            def load_wt(W, name, dst_pool, wstage_pool, tr_ps, tr_tag):
                tiles = [
                    dst_pool.tile([P, E], f32r, name=f"{name}T_{c}")
                    for c in range(EC)
                ]
                for rb in range(2):
                    wnats = []
                    for r4 in range(4):
                        r = rb * 4 + r4
                        wnat = wstage_pool.tile([P, E], f32, name=f"wstage_{r4}")
                        nc.sync.dma_start(wnat[:], W[r * P : (r + 1) * P, :])
                        wnats.append(wnat)
                    for c in range(EC):
                        pt = tr_ps.tile([P, 512], f32, name=tr_tag)
                        for r4 in range(4):
                            nc.tensor.transpose(
                                pt[:, r4 * P : (r4 + 1) * P],
                                wnats[r4][:, c * P : (c + 1) * P],
                                ident[:],
                            )
                        nc.vector.tensor_copy(
                            tiles[c][:, rb * 512 : (rb + 1) * 512], pt[:]
                        )
                return tiles

            bd_st = []
            for pr in range(8):
                s_t = small.tile([P, P], f32, name=f"bd_st_{pr}")
                nc.vector.memset(s_t[:], 0.0)
                bd_st.append(s_t)

            # ================= PHASE A ====================================
            with (
                tc.tile_pool(name="wts_kv", bufs=1) as wts_kv,
                tc.tile_pool(name="a_in", bufs=cfg.get("a_in", 2)) as a_in,
                tc.tile_pool(name="a_xt", bufs=cfg.get("a_xt", 2)) as a_xt,
                tc.tile_pool(name="a_act", bufs=cfg.get("a_act", 2)) as a_act,
                tc.tile_pool(name="a_tr_ps", bufs=cfg.get("a_tr_ps", 2), space="PSUM") as a_tr_ps,
                tc.tile_pool(name="a_pj_ps", bufs=cfg.get("a_pj_ps", 4), space="PSUM") as a_pj_ps,
                tc.tile_pool(name="a_sc_ps", bufs=1, space="PSUM") as a_sc_ps,
            ):
                with tc.tile_pool(name="wstage_a", bufs=1) as wstage_a:
                    WkT = load_wt(Wk, "Wk", wts_kv, wstage_a, a_tr_ps, "a_tr")
                    WvT = load_wt(Wv, "Wv", wts_kv, wstage_a, a_tr_ps, "a_tr")

                scores_ps = a_sc_ps.tile([P, H * D], f32, name="scores_ps")
                for qtr in range(8):
                    nc.tensor.matmul(
                        scores_ps[:, qtr * P : (qtr + 1) * P],
                        zero128[:],
                        zero128[:],
                        start=True,
                        stop=True,
                        skip_group_check=True,
                    )

                def load_pair(x_src, it2, name):
                    """One 1MB DMA covering two 128-row s-tiles."""
                    xnat2 = a_in.tile([P, 2 * E], f32, name=f"{name}_nat")
                    nc.sync.dma_start(
                        xnat2[:].rearrange("p (t e) -> p t e", t=2),
                        x_src[it2 * 2 * P : (it2 + 1) * 2 * P, :].rearrange(
                            "(t p) e -> p t e", p=P
                        ),
                    )
                    return xnat2

                def transpose_in(xnat2, sub, name, on_act=False):
                    xt = a_xt.tile([P, E], f32r, name=f"{name}_T")
                    for h in range(2):
                        pt = a_tr_ps.tile([P, 512], f32, name="a_tr")
                        for c4 in range(4):
                            c = h * 4 + c4
                            nc.tensor.transpose(
                                pt[:, c4 * P : (c4 + 1) * P],
                                xnat2[:, sub * E + c * P : sub * E + (c + 1) * P],
                                ident[:],
                            )
                        dst = xt[:, h * 512 : (h + 1) * 512]
                        if on_act:
                            nc.scalar.copy(dst, pt[:])
                        else:
                            nc.vector.tensor_copy(dst, pt[:])
                    return xt

                def project_nat(xt, WT, brow, name):
                    halves = []
                    for h in range(2):
                        pj = a_pj_ps.tile([P, 512], f32, name="pj")
                        for c in range(EC):
                            nc.tensor.matmul(
                                pj[:],
                                xt[:, c * P : (c + 1) * P],
                                WT[c][:, h * 512 : (h + 1) * 512],
                                start=(c == 0),
                                stop=(brow is None and c == EC - 1),
                            )
                        if brow is not None:
                            nc.tensor.matmul(
                                pj[:],
                                ones_1x128[:],
                                brow[:, h * 512 : (h + 1) * 512],
                                start=False,
                                stop=True,
                            )
                        halves.append(pj)
                    return halves

                for it in range(ST):
                    if it % 2 == 0:
                        xk_nat2 = load_pair(xk, it // 2, "xk")
                        xv_nat2 = load_pair(xv, it // 2, "xv")
                    xkT = transpose_in(xk_nat2, it % 2, "xk")
                    kp = project_nat(xkT, WkT, bk_row, "k")
                    # per-head interleave: head hh at cols [128*hh, 128*hh+128),
                    # v in the low 64, k(elu) in the high 64
                    kv_sb = a_act.tile([P, 2 * E], f32, name="kv_sb")
                    kv4 = kv_sb[:].rearrange("p (hh two) -> p hh two", two=2 * D)
                    for h in range(2):
                        r_sb = a_act.tile([P, 512], f32, name="kr_sb")
                        t_sb = a_act.tile([P, 512], f32, name="kt_sb")
                        e_sb = a_act.tile([P, 512], f32, name="ke_sb")
                        nc.scalar.activation(r_sb[:], kp[h][:], AF.Relu)
                        # elu(x) = relu(x) + min(exp(x), 1) - 1
                        nc.scalar.activation(e_sb[:], kp[h][:], AF.Exp)
                        nc.vector.tensor_scalar(
                            t_sb[:], e_sb[:], 1.0, -1.0, ALU.min, ALU.add
                        )
                        nc.vector.tensor_tensor(
                            kv4[:, 8 * h : 8 * (h + 1), D : 2 * D],
                            t_sb[:].rearrange("p (hh d) -> p hh d", d=D),
                            r_sb[:].rearrange("p (hh d) -> p hh d", d=D),
                            ALU.add,
                        )

                    xvT = transpose_in(xv_nat2, it % 2, "xv", on_act=True)
                    vp = project_nat(xvT, WvT, bv_row, "v")
                    for h in range(2):
                        nc.scalar.copy(
                            kv4[:, 8 * h : 8 * (h + 1), 0:D],
                            vp[h][:].rearrange("p (hh d) -> p hh d", d=D),
                        )

                    for hh in range(H):
                        nc.tensor.matmul(
                            scores_ps[:, hh * D : (hh + 1) * D],
                            kv_sb[:, 2 * D * hh : 2 * D * (hh + 1)],
                            kv_sb[:, 2 * D * hh + D : 2 * D * (hh + 1)],
                            start=False,
                            stop=(it == ST - 1 and hh == H - 1),
                            skip_group_check=True,
                        )

                # -- extract scoresT + ksumsq while phase-A psum still alive
                # Gram rows (64:128) hold k^T k per head; diagonal = ksumsq
                gram_sb = small.tile([D, H * D], f32, name="gram_sb")
                nc.vector.tensor_copy(gram_sb[:], scores_ps[D:P, :])
                gram_dram = drpool.tile([1, D * H * D], f32, name="gram_dram")
                nc.sync.dma_start(
                    gram_dram[:].rearrange("1 (d c) -> d c", d=D), gram_sb[:]
                )
                # diag idx for (hh, d) = d*(H*D) + hh*D + d = d*(H*D+1) + D*hh
                kcol = small.tile([P, EC], f32, name="kcol")
                gd = gram_dram[:].tensor
                for h2 in range(2):
                    src_ap = bass.AP(
                        gd, h2 * D, [[H * D + 1, D], [2 * D, EC]]
                    )
                    nc.sync.dma_start(kcol[h2 * D : (h2 + 1) * D, :], src_ap)
                knorm = small.tile([P, EC], f32, name="knorm")
                nc.scalar.activation(knorm[:], kcol[:], AF.Sqrt, scale=float(S))
                invk = small.tile([P, EC], f32, name="invk")
                nc.vector.reciprocal(invk[:], knorm[:])

                bd = []
                for pr in range(8):
                    h0, h1 = 2 * pr, 2 * pr + 1
                    nc.vector.tensor_copy(
                        bd_st[pr][0:D, 0:D], scores_ps[0:D, h0 * D : (h0 + 1) * D]
                    )
                    odd_stage = small.tile([D, D], f32, name="odd_stage")
                    nc.vector.tensor_copy(
                        odd_stage[:], scores_ps[0:D, h1 * D : (h1 + 1) * D]
                    )
                    nc.sync.dma_start(bd_st[pr][D:P, D:P], odd_stage[:])
                    bd_t = small.tile([P, P], f32r, name=f"bd_{pr}")
                    nc.vector.tensor_copy(bd_t[:], bd_st[pr][:])
                    bd.append(bd_t)

            rows_scope.__exit__(None, None, None)

            # ================= PHASE B: W2 ================================
            w2scope = tc.tile_pool(name="w2pool", bufs=1)
            w2pool = w2scope.__enter__()
            W2 = [w2pool.tile([P, E], f32r, name=f"W2_{c}") for c in range(EC)]
            with (
                tc.tile_pool(name="wts_o", bufs=1) as wts_o,
                tc.tile_pool(name="b_stage", bufs=2) as b_stage,
                tc.tile_pool(name="b_tr_ps", bufs=2, space="PSUM") as b_tr_ps,
                tc.tile_pool(name="b_ps", bufs=4, space="PSUM") as b_ps,
            ):
                WoT = load_wt(Wo, "Wo", wts_o, b_stage, b_tr_ps, "b_tr")
                for c in range(EC):
                    for h in range(2):
                        w2p = b_ps.tile([P, 512], f32, name="w2_ps")
                        nc.tensor.matmul(
                            w2p[:],
                            bd[c][:],
                            WoT[c][:, h * 512 : (h + 1) * 512],
                            start=True,
                            stop=True,
                        )
                        nc.vector.tensor_scalar(
                            W2[c][:, h * 512 : (h + 1) * 512],
                            w2p[:],
                            invk[:, c : c + 1],
                            None,
                            ALU.mult,
                        )

            # ================= PHASE C: q pass ============================
            with (
                tc.tile_pool(name="wts_q", bufs=1) as wts_q,
                tc.tile_pool(name="c_in", bufs=1) as c_in,
                tc.tile_pool(name="c_xt", bufs=cfg.get("c_xt", 1)) as c_xt,
                tc.tile_pool(name="c_qt", bufs=cfg.get("c_qt", 1)) as c_qt,
                tc.tile_pool(name="c_tmp", bufs=cfg.get("c_tmp", 2)) as c_tmp,
                tc.tile_pool(name="c_out", bufs=cfg.get("c_out", 1)) as c_out,
                tc.tile_pool(name="c_tr_ps", bufs=cfg.get("c_tr_ps", 2), space="PSUM") as c_tr_ps,
                tc.tile_pool(name="c_pj_ps", bufs=cfg.get("c_pj_ps", 2), space="PSUM") as c_pj_ps,
                tc.tile_pool(name="c_ss_ps", bufs=1, space="PSUM") as c_ss_ps,
                tc.tile_pool(name="c_qb_ps", bufs=1, space="PSUM") as c_qb_ps,
                tc.tile_pool(name="c_fin_ps", bufs=cfg.get("c_fin_ps", 2), space="PSUM") as c_fin_ps,
            ):
                with tc.tile_pool(name="wstage_c", bufs=1) as wstage_c:
                    WqT = load_wt(Wq, "Wq", wts_q, wstage_c, c_tr_ps, "c_tr")

                for blk_i in range(NBLK):
                    s0 = blk_i * SBLK
                    xqT = [
                        c_xt.tile([P, SBLK], f32r, name=f"xqT_{c}")
                        for c in range(EC)
                    ]
                    xq_blk = c_in.tile([P, JB * E], f32, name="xq_blk")
                    nc.sync.dma_start(
                        xq_blk[:].rearrange("p (t e) -> p t e", t=JB),
                        xq[s0 : s0 + SBLK, :].rearrange("(t p) e -> p t e", p=P),
                    )
                    for c in range(EC):
                        pt = c_tr_ps.tile([P, 512], f32, name="c_tr")
                        for j in range(JB):
                            nc.tensor.transpose(
                                pt[:, j * P : (j + 1) * P],
                                xq_blk[:, j * E + c * P : j * E + (c + 1) * P],
                                ident[:],
                            )
                        if c % 2 == 0:
                            nc.vector.tensor_copy(xqT[c][:], pt[:])
                        else:
                            nc.scalar.copy(xqT[c][:], pt[:])

                    qss_ps = c_ss_ps.tile([H, SBLK], f32, name="qss_ps")
                    qt_tiles = []
                    for ot in range(EC):
                        pj = c_pj_ps.tile([P, SBLK], f32, name="q_pj")
                        for c in range(EC):
                            nc.tensor.matmul(
                                pj[:],
                                WqT[c][:, ot * P : (ot + 1) * P],
                                xqT[c][:],
                                start=(c == 0),
                                stop=(c == EC - 1),
                            )
                        r_sb = c_tmp.tile([P, SBLK], f32, name="qr_sb")
                        t_sb = c_tmp.tile([P, SBLK], f32, name="qt_sb")
                        e_sb = c_tmp.tile([P, SBLK], f32, name="qe_sb")
                        qt_ = c_qt.tile([P, SBLK], f32, name=f"qt_{ot}")
                        qbias = bq_col[:, ot : ot + 1] if with_bias else 0.0
                        nc.scalar.activation(r_sb[:], pj[:], AF.Relu, bias=qbias)
                        # elu(x) = relu(x) + min(exp(x), 1) - 1
                        nc.scalar.activation(e_sb[:], pj[:], AF.Exp, bias=qbias)
                        nc.vector.tensor_scalar(
                            t_sb[:], e_sb[:], 1.0, -1.0, ALU.min, ALU.add
                        )
                        nc.vector.tensor_tensor(
                            qt_[:], t_sb[:], r_sb[:], ALU.add
                        )
                        qt_tiles.append(qt_)
                        q2 = c_tmp.tile([P, SBLK], f32r, name="q2_sb")
                        nc.scalar.activation(q2[:], qt_[:], AF.Square)
                        nc.tensor.matmul(
                            qss_ps[:],
                            blockones[ot][:],
                            q2[:],
                            start=(ot == 0),
                            stop=(ot == EC - 1),
                        )

                    qss_sb = c_tmp.tile([H, SBLK], f32, name="qss_sb")
                    nc.scalar.activation(qss_sb[:], qss_ps[:], AF.Sqrt,
                                         scale=float(D))
                    invq = c_tmp.tile([H, SBLK], f32, name="invq")
                    nc.vector.reciprocal(invq[:], qss_sb[:])
                    invq_r = c_tmp.tile([H, SBLK], f32r, name="invq_r")
                    nc.vector.tensor_copy(invq_r[:], invq[:])

                    # broadcast + in-place scale (qt tile becomes f32r qn)
                    qn_tiles = []
                    for ot in range(EC):
                        qb = c_qb_ps.tile([P, SBLK], f32, name="qb_ps")
                        nc.tensor.matmul(
                            qb[:], blockpick[ot][:], invq_r[:],
                            start=True, stop=True,
                        )
                        qn = c_qt.tile([P, SBLK], f32r, name=f"qn_{ot}")
                        nc.vector.tensor_tensor(
                            qn[:], qt_tiles[ot][:], qb[:], ALU.mult
                        )
                        qn_tiles.append(qn)

                    for j2 in range(JB // 2):
                        o_sb = c_out.tile([P, 2 * E], f32, name="o_sb")
                        for tj in range(2):
                            j = j2 * 2 + tj
                            for h in range(2):
                                fin = c_fin_ps.tile([P, 512], f32, name="fin_ps")
                                for c in range(EC):
                                    nc.tensor.matmul(
                                        fin[:],
                                        qn_tiles[c][:, j * P : (j + 1) * P],
                                        W2[c][:, h * 512 : (h + 1) * 512],
                                        start=(c == 0),
                                        stop=(c == EC - 1),
                                    )
                                sl = slice(tj * E + h * 512, tj * E + (h + 1) * 512)
                                if with_bias:
                                    nc.vector.scalar_tensor_tensor(
                                        o_sb[:, sl], fin[:], 0.0,
                                        bo_bcast[:, h * 512 : (h + 1) * 512],
                                        ALU.add, ALU.add,
                                    )
                                else:
                                    nc.vector.tensor_copy(o_sb[:, sl], fin[:])
                        nc.sync.dma_start(
                            out[s0 + j2 * 2 * P : s0 + (j2 + 1) * 2 * P, :]
                            .rearrange("(t p) e -> p t e", p=P),
                            o_sb[:].rearrange("p (t e) -> p t e", t=2),
                        )
            w2scope.__exit__(None, None, None)

    _patch_bass(nc)
    return nc


# --------------------------------------------------------------------------
# Host wrapper
# --------------------------------------------------------------------------
_NC_CACHE = {}


def _get_nc(S, with_bias=True):
    key = (S, with_bias)
    if key not in _NC_CACHE:
        _NC_CACHE[key] = build(S, with_bias)
    return _NC_CACHE[key]


def make_in_maps(query, key, value, Wq, bq, Wk, bk, Wv, bv, Wo, bo):
    query = np.asarray(query, np.float32)
    key = np.asarray(key, np.float32)
    value = np.asarray(value, np.float32)
    B = query.shape[0]
    shared = {
        "Wq": np.ascontiguousarray(np.asarray(Wq, np.float32)),
        "Wk": np.ascontiguousarray(np.asarray(Wk, np.float32)),
        "Wv": np.ascontiguousarray(np.asarray(Wv, np.float32)),
        "Wo": np.ascontiguousarray(np.asarray(Wo, np.float32)),
        "bq": np.ascontiguousarray(np.asarray(bq, np.float32).reshape(1, E)),
        "bk": np.ascontiguousarray(np.asarray(bk, np.float32).reshape(1, E)),
        "bv": np.ascontiguousarray(np.asarray(bv, np.float32).reshape(1, E)),
        "bo": np.ascontiguousarray(np.asarray(bo, np.float32).reshape(1, E)),
    }
    return [
        {
            "xq": np.ascontiguousarray(query[c]),
            "xk": np.ascontiguousarray(key[c]),
            "xv": np.ascontiguousarray(value[c]),
            **shared,
        }
        for c in range(B)
    ]


def kernel(query, key, value, Wq, bq, Wk, bk, Wv, bv, Wo, bo):
    query = np.asarray(query, np.float32)
    B, S, E_ = query.shape
    assert E_ == E and B == N_CORES
    in_maps = make_in_maps(query, key, value, Wq, bq, Wk, bk, Wv, bv, Wo, bo)
    with_bias = any(
        np.any(np.asarray(b)) for b in (bq, bk, bv, bo)
    )
    nc = _get_nc(S, with_bias)
    res = run_bass_kernel_spmd(nc, in_maps, core_ids=list(range(N_CORES)))
    return np.stack([res.results[c]["out"] for c in range(B)])
